# revision 7
# baseline (speedup 1.0000x reference)
"""MoE routing kernel for Trainium2 (8 NeuronCores, Bass/Tile).

Problem: B=32768 tokens, IN=HID=C=128, E=32 experts.
    scores = softmax(relu(x @ Wg + bg))                      [B, E]
    h      = relu(einsum('bi,eih->beh', x, W1) + b1)         [B, E, HID]
    eo     = einsum('beh,eho->beo', h, W2) + b2              [B, E, HID]
    out    = einsum('beh,hc->bec', eo, Wc) + bc              [B, E, C]

Strategy:
  - Data parallel: batch sharded across 8 cores (4096 tokens each),
    expert weights replicated.
  - Algebraic fusion: out = h @ (W2[e] @ Wc) + (b2[e] @ Wc + bc), so only
    two big matmuls per (token, expert) instead of three.  W2c and bc' are
    tiny (E*HID*C) and precomputed on the host along with layout transforms.
  - Per core: loop over 8 tiles of 512 tokens; per tile transpose x once
    (PE transpose), then per expert: mm1 -> relu+b1 (ScalarE, per-partition
    bias) -> mm2 chunks -> VectorE add of broadcast bc' while copying into
    a [128, 4*32*128] staging buffer laid out so the 8 MB tile store is one
    fully-contiguous DMA.
  - bf16 matmul operands (fp32 PSUM accumulate); fp32 everywhere else.
"""

import numpy as np
import ml_dtypes

B, IN, HID, E, C = 32768, 128, 128, 32, 128
NCORES = 8
BS = B // NCORES        # tokens per core
TILE = 512              # tokens per tile
NTILES = BS // TILE
NCH = TILE // 128       # 128-token chunks per tile

BF16 = ml_dtypes.bfloat16

_NC_CACHE = {}


def _build_nc():
    import concourse.bass as bass
    import concourse.bacc as bacc
    import concourse.tile as tile
    import concourse.mybir as mybir
    from concourse.bass import ts
    from concourse.masks import make_identity

    fp32 = mybir.dt.float32
    bf16 = mybir.dt.bfloat16
    AF = mybir.ActivationFunctionType

    nc = bacc.Bacc("TRN2", target_bir_lowering=False, debug=False)

    xs = nc.dram_tensor("xs", [BS, IN], fp32, kind="ExternalInput")
    w1t = nc.dram_tensor("w1t", [IN, E, HID], bf16, kind="ExternalInput")
    w2ct = nc.dram_tensor("w2ct", [HID, E, C], bf16, kind="ExternalInput")
    wgd = nc.dram_tensor("wgd", [IN, E], bf16, kind="ExternalInput")
    b1t = nc.dram_tensor("b1t", [HID, E], fp32, kind="ExternalInput")
    bgb = nc.dram_tensor("bgb", [1, NCH * E], fp32, kind="ExternalInput")
    bcb = nc.dram_tensor("bcb", [1, E * C], fp32, kind="ExternalInput")
    out = nc.dram_tensor("out", [BS, E, C], fp32, kind="ExternalOutput")
    sco = nc.dram_tensor("sco", [BS, E], fp32, kind="ExternalOutput")

    def bcast_ap(src_dram, parts=128):
        return bass.AP(
            tensor=src_dram.tensor if isinstance(src_dram, bass.AP) else src_dram,
            offset=0,
            ap=[[0, parts]] + [list(p) for p in src_dram[:].ap[1:]],
        )

    with tile.TileContext(nc) as tc:
        with (
            tc.tile_pool(name="consts", bufs=1) as consts,
            tc.tile_pool(name="xin", bufs=2) as xin,
            tc.tile_pool(name="xtp", bufs=2) as xtp,
            tc.tile_pool(name="hpool", bufs=3) as hpool,
            tc.tile_pool(name="stage", bufs=2) as stage,
            tc.tile_pool(name="scstage", bufs=2) as scstage,
            tc.tile_pool(name="small", bufs=4) as small,
            tc.tile_pool(name="ps_h", bufs=2, space="PSUM") as ps_h,
            tc.tile_pool(name="ps_o", bufs=2, space="PSUM") as ps_o,
            tc.tile_pool(name="ps_x", bufs=2, space="PSUM") as ps_x,
            tc.tile_pool(name="ps_r", bufs=2, space="PSUM") as ps_r,
        ):
            # ---- constants / weights (loaded once) ----
            w1sb = consts.tile([IN, E, HID], bf16)
            nc.scalar.dma_start(out=w1sb, in_=w1t[:])
            w2csb = consts.tile([HID, E, C], bf16)
            nc.scalar.dma_start(out=w2csb, in_=w2ct[:])
            wgsb = consts.tile([IN, E], bf16)
            nc.scalar.dma_start(out=wgsb, in_=wgd[:])
            b1sb = consts.tile([HID, E], fp32)
            nc.scalar.dma_start(out=b1sb, in_=b1t[:])
            # broadcast bias tiles (replicated across all 128 partitions)
            bgsb = consts.tile([128, NCH * E], fp32)
            nc.gpsimd.dma_start(out=bgsb, in_=bcast_ap(bgb))
            bcsb = consts.tile([128, E, C], fp32)
            nc.gpsimd.dma_start(
                out=bcsb.rearrange("p e c -> p (e c)"), in_=bcast_ap(bcb)
            )
            ident = consts.tile([128, 128], fp32)
            make_identity(nc, ident)

            for t in range(NTILES):
                # ---- load x tile: (p, g, i) <- xs[t*TILE + g*128 + p, i] ----
                xt_in = xin.tile([128, NCH, IN], fp32)
                nc.scalar.dma_start(
                    out=xt_in,
                    in_=xs[ts(t, TILE), :].rearrange("(g p) i -> p g i", p=128),
                )

                # ---- transpose to xT [IN, TILE] bf16 via PE ----
                xT = xtp.tile([IN, TILE], bf16)
                for g in range(NCH):
                    pst = ps_x.tile([128, 128], fp32)
                    nc.tensor.transpose(pst, xt_in[:, g, :], ident)
                    nc.vector.tensor_copy(out=xT[:, ts(g, 128)], in_=pst)

                # ---- router: scores = softmax(relu(x @ Wg + bg)) ----
                ps_sc = ps_r.tile([128, NCH, E], fp32)
                for g in range(NCH):
                    nc.tensor.matmul(
                        ps_sc[:, g, :], xT[:, ts(g, 128)], wgsb,
                        start=True, stop=True,
                    )
                lg = small.tile([128, NCH, E], fp32)
                nc.vector.tensor_add(
                    lg, ps_sc, bgsb.rearrange("p (g e) -> p g e", g=NCH)
                )
                nc.vector.tensor_scalar_max(lg, lg, 0.0)
                es = small.tile([128, NCH, E], fp32)
                nc.scalar.activation(es, lg, AF.Exp)
                ssum = small.tile([128, NCH], fp32)
                nc.vector.reduce_sum(ssum, es, axis=mybir.AxisListType.X)
                rcp = small.tile([128, NCH], fp32)
                nc.vector.reciprocal(rcp, ssum)
                scstg = scstage.tile([128, NCH, E], fp32)
                for g in range(NCH):
                    nc.vector.tensor_scalar_mul(
                        scstg[:, g, :], es[:, g, :], rcp[:, ts(g, 1)]
                    )
                nc.sync.dma_start(
                    out=sco[ts(t, TILE), :].rearrange("(g p) e -> p g e", p=128),
                    in_=scstg,
                )

                # ---- experts ----
                stg = stage.tile([128, NCH, E, C], fp32)
                for e in range(E):
                    ps_ht = ps_h.tile([HID, TILE], fp32)
                    nc.tensor.matmul(ps_ht, w1sb[:, e, :], xT, start=True, stop=True)
                    h = hpool.tile([HID, TILE], bf16)
                    nc.scalar.activation(
                        h, ps_ht, AF.Relu, bias=b1sb[:, ts(e, 1)], scale=1.0
                    )
                    ps_ot = ps_o.tile([128, NCH, C], fp32)
                    for g in range(NCH):
                        nc.tensor.matmul(
                            ps_ot[:, g, :], h[:, ts(g, 128)], w2csb[:, e, :],
                            start=True, stop=True,
                        )
                    # stg[:, g, e, :] = ps_ot[:, g, :] + bc'(e) (g-broadcast)
                    bcB = bcsb[:, e, :]
                    bc_in = bass.AP(
                        tensor=bcB.tensor,
                        offset=bcB.offset,
                        ap=[list(bcB.ap[0]), [0, NCH], list(bcB.ap[1])],
                    )
                    nc.vector.tensor_add(stg[:, :, e, :], ps_ot, bc_in)

                nc.sync.dma_start(
                    out=out[ts(t, TILE), :, :].rearrange(
                        "(g p) e c -> p g e c", p=128
                    ),
                    in_=stg,
                )
    nc.compile()
    return nc


def _get_nc():
    if "nc" not in _NC_CACHE:
        _NC_CACHE["nc"] = _build_nc()
    return _NC_CACHE["nc"]


def make_prep(Wg, bg, W1, b1, W2, b2, Wc, bc):
    W2c = np.matmul(W2, Wc)                     # [E, HID, C]
    bcp = b2 @ Wc + bc                          # [E, C]
    return {
        "w1t": np.ascontiguousarray(W1.transpose(1, 0, 2)).astype(BF16),    # [IN, E, HID]
        "w2ct": np.ascontiguousarray(W2c.transpose(1, 0, 2)).astype(BF16),  # [HID, E, C]
        "wgd": Wg.astype(BF16),                                             # [IN, E]
        "b1t": np.ascontiguousarray(b1.T).astype(np.float32),               # [HID, E]
        "bgb": np.tile(bg, NCH)[None, :].astype(np.float32),                # [1, NCH*E]
        "bcb": bcp.reshape(1, -1).astype(np.float32),                       # [1, E*C]
    }


def kernel(x, Wg, bg, W1, b1, W2, b2, Wc, bc):
    from concourse.bass_utils import run_bass_kernel_spmd

    x = np.ascontiguousarray(np.asarray(x, dtype=np.float32))
    prep = make_prep(
        *(np.asarray(a, dtype=np.float32) for a in (Wg, bg, W1, b1, W2, b2, Wc, bc))
    )
    in_maps = [
        {**prep, "xs": x[i * BS : (i + 1) * BS]} for i in range(NCORES)
    ]

    nc = _get_nc()
    res = run_bass_kernel_spmd(nc, in_maps, core_ids=list(range(NCORES)))
    out = np.concatenate([r["out"] for r in res.results], axis=0)
    scores = np.concatenate([r["sco"] for r in res.results], axis=0)
    return out, scores


# revision 9
# speedup vs baseline: 1.1940x; 1.1940x over previous
"""MoE routing kernel for Trainium2 (8 NeuronCores, Bass/Tile).

Problem: B=32768 tokens, IN=HID=C=128, E=32 experts.
    scores = softmax(relu(x @ Wg + bg))                      [B, E]
    h      = relu(einsum('bi,eih->beh', x, W1) + b1)         [B, E, HID]
    eo     = einsum('beh,eho->beo', h, W2) + b2              [B, E, HID]
    out    = einsum('beh,hc->bec', eo, Wc) + bc              [B, E, C]

Strategy:
  - Data parallel: batch sharded across 8 cores (4096 tokens each),
    expert weights replicated.
  - Algebraic fusion: out = h @ (W2[e] @ Wc) + (b2[e] @ Wc + bc), so only
    two big matmuls per (token, expert) instead of three.  W2c and bc' are
    tiny (E*HID*C) and precomputed on the host along with layout transforms.
  - Per core: loop over 8 tiles of 512 tokens; per tile transpose x once
    (PE transpose), then per expert: mm1 -> relu+b1 (ScalarE, per-partition
    bias) -> mm2 chunks -> VectorE add of broadcast bc' while copying into
    a [128, 4*32*128] staging buffer laid out so the 8 MB tile store is one
    fully-contiguous DMA.
  - bf16 matmul operands (fp32 PSUM accumulate); fp32 everywhere else.
"""

import numpy as np
import ml_dtypes

B, IN, HID, E, C = 32768, 128, 128, 32, 128
NCORES = 8
BS = B // NCORES        # tokens per core
TILE = 512              # tokens per tile
NTILES = BS // TILE
NCH = TILE // 128       # 128-token chunks per tile

BF16 = ml_dtypes.bfloat16

_NC_CACHE = {}


def _build_nc():
    import concourse.bass as bass
    import concourse.bacc as bacc
    import concourse.tile as tile
    import concourse.mybir as mybir
    from concourse.bass import ts
    from concourse.masks import make_identity

    fp32 = mybir.dt.float32
    bf16 = mybir.dt.bfloat16
    AF = mybir.ActivationFunctionType

    nc = bacc.Bacc("TRN2", target_bir_lowering=False, debug=False)

    xs = nc.dram_tensor("xs", [BS, IN], fp32, kind="ExternalInput")
    w1t = nc.dram_tensor("w1t", [IN, E, HID], bf16, kind="ExternalInput")
    w2ct = nc.dram_tensor("w2ct", [HID, E, C], bf16, kind="ExternalInput")
    wgd = nc.dram_tensor("wgd", [IN, E], bf16, kind="ExternalInput")
    b1t = nc.dram_tensor("b1t", [HID, E], fp32, kind="ExternalInput")
    bgb = nc.dram_tensor("bgb", [1, NCH * E], fp32, kind="ExternalInput")
    bcb = nc.dram_tensor("bcb", [1, E * C], fp32, kind="ExternalInput")
    out = nc.dram_tensor("out", [BS, E, C], fp32, kind="ExternalOutput")
    sco = nc.dram_tensor("sco", [BS, E], fp32, kind="ExternalOutput")

    def bcast_ap(src_dram, parts=128):
        return bass.AP(
            tensor=src_dram.tensor if isinstance(src_dram, bass.AP) else src_dram,
            offset=0,
            ap=[[0, parts]] + [list(p) for p in src_dram[:].ap[1:]],
        )

    with tile.TileContext(nc) as tc:
        with (
            tc.tile_pool(name="consts", bufs=1) as consts,
            tc.tile_pool(name="xin", bufs=2) as xin,
            tc.tile_pool(name="xtp", bufs=2) as xtp,
            tc.tile_pool(name="hpool", bufs=3) as hpool,
            tc.tile_pool(name="stage", bufs=2) as stage,
            tc.tile_pool(name="scstage", bufs=2) as scstage,
            tc.tile_pool(name="small", bufs=4) as small,
            tc.tile_pool(name="ps_h", bufs=3, space="PSUM") as ps_h,
            tc.tile_pool(name="ps_o", bufs=3, space="PSUM") as ps_o,
            tc.tile_pool(name="ps_x", bufs=1, space="PSUM") as ps_x,
            tc.tile_pool(name="ps_r", bufs=1, space="PSUM") as ps_r,
        ):
            # ---- constants / weights (loaded once) ----
            w1sb = consts.tile([IN, E, HID], bf16)
            nc.scalar.dma_start(out=w1sb, in_=w1t[:])
            w2csb = consts.tile([HID, E, C], bf16)
            nc.scalar.dma_start(out=w2csb, in_=w2ct[:])
            wgsb = consts.tile([IN, E], bf16)
            nc.scalar.dma_start(out=wgsb, in_=wgd[:])
            b1sb = consts.tile([HID, E], fp32)
            nc.scalar.dma_start(out=b1sb, in_=b1t[:])
            # broadcast bias tiles (replicated across all 128 partitions)
            bgsb = consts.tile([128, NCH * E], fp32)
            nc.gpsimd.dma_start(out=bgsb, in_=bcast_ap(bgb))
            bcsb = consts.tile([128, E, C], fp32)
            nc.gpsimd.dma_start(
                out=bcsb.rearrange("p e c -> p (e c)"), in_=bcast_ap(bcb)
            )
            ident = consts.tile([128, 128], fp32)
            make_identity(nc, ident)

            for t in range(NTILES):
                # ---- load x tile: (p, g, i) <- xs[t*TILE + g*128 + p, i] ----
                xt_in = xin.tile([128, NCH, IN], fp32)
                nc.scalar.dma_start(
                    out=xt_in,
                    in_=xs[ts(t, TILE), :].rearrange("(g p) i -> p g i", p=128),
                )

                # ---- transpose to xT [IN, TILE] bf16 via PE ----
                xT = xtp.tile([IN, TILE], bf16)
                for g in range(NCH):
                    pst = ps_x.tile([128, 128], fp32)
                    nc.tensor.transpose(pst, xt_in[:, g, :], ident)
                    nc.vector.tensor_copy(out=xT[:, ts(g, 128)], in_=pst)

                # ---- router: scores = softmax(relu(x @ Wg + bg)) ----
                ps_sc = ps_r.tile([128, NCH, E], fp32)
                for g in range(NCH):
                    nc.tensor.matmul(
                        ps_sc[:, g, :], xT[:, ts(g, 128)], wgsb,
                        start=True, stop=True,
                    )
                lg = small.tile([128, NCH, E], fp32)
                nc.vector.tensor_add(
                    lg, ps_sc, bgsb.rearrange("p (g e) -> p g e", g=NCH)
                )
                nc.vector.tensor_scalar_max(lg, lg, 0.0)
                es = small.tile([128, NCH, E], fp32)
                nc.scalar.activation(es, lg, AF.Exp)
                ssum = small.tile([128, NCH], fp32)
                nc.vector.reduce_sum(ssum, es, axis=mybir.AxisListType.X)
                rcp = small.tile([128, NCH], fp32)
                nc.vector.reciprocal(rcp, ssum)
                scstg = scstage.tile([128, NCH, E], fp32)
                for g in range(NCH):
                    nc.vector.tensor_scalar_mul(
                        scstg[:, g, :], es[:, g, :], rcp[:, ts(g, 1)]
                    )
                nc.sync.dma_start(
                    out=sco[ts(t, TILE), :].rearrange("(g p) e -> p g e", p=128),
                    in_=scstg,
                )

                # ---- experts ----
                stg = stage.tile([128, NCH, E, C], fp32)
                for e in range(E):
                    ps_ht = ps_h.tile([HID, TILE], fp32)
                    nc.tensor.matmul(ps_ht, w1sb[:, e, :], xT, start=True, stop=True)
                    h = hpool.tile([HID, TILE], bf16)
                    nc.scalar.activation(
                        h, ps_ht, AF.Relu, bias=b1sb[:, ts(e, 1)], scale=1.0
                    )
                    ps_ot = ps_o.tile([128, NCH, C], fp32)
                    for g in range(NCH):
                        nc.tensor.matmul(
                            ps_ot[:, g, :], h[:, ts(g, 128)], w2csb[:, e, :],
                            start=True, stop=True,
                        )
                    # stg[:, g, e, :] = ps_ot[:, g, :] + bc'(e) (g-broadcast)
                    bcB = bcsb[:, e, :]
                    bc_in = bass.AP(
                        tensor=bcB.tensor,
                        offset=bcB.offset,
                        ap=[list(bcB.ap[0]), [0, NCH], list(bcB.ap[1])],
                    )
                    nc.vector.tensor_add(stg[:, :, e, :], ps_ot, bc_in)

                    # store each quarter of the experts as soon as it's done,
                    # so the final tile's tail DMA is 2 MB instead of 8 MB
                    if e % 8 == 7:
                        eb = e - 7
                        nc.sync.dma_start(
                            out=out[ts(t, TILE), :, :].rearrange(
                                "(g p) e c -> p g e c", p=128
                            )[:, :, eb : eb + 8, :],
                            in_=stg[:, :, eb : eb + 8, :],
                        )
    nc.compile()
    return nc


def _get_nc():
    if "nc" not in _NC_CACHE:
        _NC_CACHE["nc"] = _build_nc()
    return _NC_CACHE["nc"]


def make_prep(Wg, bg, W1, b1, W2, b2, Wc, bc):
    W2c = np.matmul(W2, Wc)                     # [E, HID, C]
    bcp = b2 @ Wc + bc                          # [E, C]
    return {
        "w1t": np.ascontiguousarray(W1.transpose(1, 0, 2)).astype(BF16),    # [IN, E, HID]
        "w2ct": np.ascontiguousarray(W2c.transpose(1, 0, 2)).astype(BF16),  # [HID, E, C]
        "wgd": Wg.astype(BF16),                                             # [IN, E]
        "b1t": np.ascontiguousarray(b1.T).astype(np.float32),               # [HID, E]
        "bgb": np.tile(bg, NCH)[None, :].astype(np.float32),                # [1, NCH*E]
        "bcb": bcp.reshape(1, -1).astype(np.float32),                       # [1, E*C]
    }


def kernel(x, Wg, bg, W1, b1, W2, b2, Wc, bc):
    from concourse.bass_utils import run_bass_kernel_spmd

    x = np.ascontiguousarray(np.asarray(x, dtype=np.float32))
    prep = make_prep(
        *(np.asarray(a, dtype=np.float32) for a in (Wg, bg, W1, b1, W2, b2, Wc, bc))
    )
    in_maps = [
        {**prep, "xs": x[i * BS : (i + 1) * BS]} for i in range(NCORES)
    ]

    nc = _get_nc()
    res = run_bass_kernel_spmd(nc, in_maps, core_ids=list(range(NCORES)))
    out = np.concatenate([r["out"] for r in res.results], axis=0)
    scores = np.concatenate([r["sco"] for r in res.results], axis=0)
    return out, scores
